# revision 48
# baseline (speedup 1.0000x reference)
"""TRN2 Bass kernel for nn_AlignHead (deformable conv 3x3 + ReLU + 1x1 cls).

Strategy: pure data parallel over batch (2 images per NeuronCore, 8 cores).
On-device pipeline per image:
  1. offsets -> patch tile indices + bilinear corner weights on DVE
     (floor via the IEEE magic-number rint trick; out-of-bounds corners get
     zero weight, matching the reference's clamp+mask semantics).
  2. cast x to f16, PE-transpose to xT [HW, C], then build 4 parity-aligned
     2x2-tiled copies of the image (TC): for any integer corner (y0, x0) the
     bilinear 2x2 patch is ONE contiguous 2KB element in copy (y0&1, x0&1),
     so one gather descriptor fetches all 4 corners.
  3. per (tap, 1024-position block): ONE bulk dma_gather (1024 int16
     indices, replicated into all 8 16-partition groups for the Q7 core
     pairs; ~1.3us of Pool DGE time vs ~8us for 8 indirect_dma_starts),
     DVE bilinear interp (1 broadcast mul in the DVE 2x perf mode + 2
     paired adds), PE transpose back to channel-major, PE conv
     accumulation over the 9 taps in PSUM.
  4. fused ReLU eviction, 1x1 cls matmul on PE, bias add, DMA out.

Scheduling: image 0's TC build forms the pipeline head (expansion DMAs
split across both HWDGE rings, first y-half issued after 16 of the 32
xT stores); image i+1's x load/transpose runs inside image i's head and
its 17.5MB expansion hides under image i's tap compute. The steady state
is jointly DMA- (~550us of SDMA busy) and DVE-bound (~590us interp).

Known HW/CoreSim divergence: PSUM accumulation across PE *transpose*
matmuls (start/stop over is_transpose=True) works in CoreSim but NOT on
silicon - the bilinear corner sum must stay on the DVE.

All gather/matmul data is fp16 with fp32 PSUM accumulation; overall
relative error vs the fp32 reference is ~7e-4.
"""

import sys
sys.path.insert(0, '/opt/trn_rl_repo')
import numpy as np

import concourse.bass as bass
import concourse.tile as tile
from concourse import bacc, mybir

f32 = mybir.dt.float32
f16 = mybir.dt.float16
i32 = mybir.dt.int32
i16 = mybir.dt.int16

N_CORES = 8
C = 256
H = W = 64
HW = H * W
NPB = 4             # position blocks per image
PB = HW // NPB      # 1024 positions per block
SL = PB // 128      # 8 slots (gathers) per block
G = 34              # tile grid per axis (m in [-1,32] stored at m+1)
NT = G * G          # tiles per parity copy
TCN = 4 * NT        # tiles per image (4 parity copies)
Ao = mybir.AluOpType
Act = mybir.ActivationFunctionType


def host_constants():
    ident = np.eye(128, dtype=np.float16)
    p = np.arange(128)
    s = np.arange(32)
    k = np.arange(9)
    pos = s[None, :, None] * 128 + p[:, None, None]
    y = pos // W
    x = pos % W
    ky = k // 3
    kx = k % 3
    byx = np.empty((128, 2, 32, 9), dtype=np.float32)
    byx[:, 0] = y + (ky[None, None, :] - 1)
    byx[:, 1] = x + (kx[None, None, :] - 1)
    return ident, byx


def pack_weights(w_def, w_cls):
    w = np.asarray(w_def, dtype=np.float32).reshape(256, 256, 9)
    wp = np.empty((128, 9, 2, 2, 128), dtype=np.float32)
    for cc in range(2):
        for oc in range(2):
            blk = w[oc * 128:(oc + 1) * 128, cc * 128:(cc + 1) * 128, :]
            wp[:, :, cc, oc, :] = blk.transpose(1, 2, 0)
    wc = np.asarray(w_cls, dtype=np.float32).reshape(256)
    wcp = wc.reshape(2, 128).T.copy()
    return wp, wcp


def build(n_img):
    nc = bacc.Bacc("TRN2", target_bir_lowering=False, debug=False,
                   num_devices=N_CORES, num_swdge_queues=4)
    x_d = nc.dram_tensor("x", [n_img, C, HW], f32, kind="ExternalInput").ap()
    off_d = nc.dram_tensor("off", [n_img, HW, 18], f32, kind="ExternalInput").ap()
    wdef_d = nc.dram_tensor("wdef", [128, 9, 2, 2, 128], f32, kind="ExternalInput").ap()
    wcls_d = nc.dram_tensor("wcls", [128, 2], f32, kind="ExternalInput").ap()
    bcls_d = nc.dram_tensor("bcls", [1, 1], f32, kind="ExternalInput").ap()
    ident_d = nc.dram_tensor("ident", [128, 128], f16, kind="ExternalInput").ap()
    byx_d = nc.dram_tensor("byx", [128, 2, 32, 9], f32, kind="ExternalInput").ap()
    out_d = nc.dram_tensor("out", [n_img, HW], f32, kind="ExternalOutput").ap()

    xT_d = nc.dram_tensor("xT", [n_img, HW, C], f16).ap()
    TC_ds = [nc.dram_tensor(f"TC{i}", [TCN, 4 * C], f16).ap()
             for i in range(n_img)]

    with tile.TileContext(nc) as tc:
        with tc.tile_pool(name="const", bufs=1) as constp:
            ident = constp.tile([128, 128], f16)
            nc.sync.dma_start(ident[:], ident_d[:])
            byx = constp.tile([128, 2, 32, 9], f32)
            nc.sync.dma_start(byx[:], byx_d[:])
            wdef = constp.tile([128, 9, 2, 2, 128], f16)
            with tc.tile_pool(name="wstage", bufs=1) as wsp:
                wst = wsp.tile([128, 9, 2, 2, 128], f32)
                nc.sync.dma_start(wst[:], wdef_d[:])
                nc.scalar.activation(wdef[:], wst[:], Act.Copy)
            wcls = constp.tile([128, 2], f16)
            wclsf = constp.tile([128, 2], f32)
            nc.sync.dma_start(wclsf[:], wcls_d[:])
            nc.vector.tensor_copy(wcls[:], wclsf[:])
            bcls = constp.tile([1, 1], f32)
            nc.sync.dma_start(bcls[:], bcls_d[:])
            zpad = constp.tile([128, 1024], f16)
            nc.vector.memset(zpad[:].rearrange("p a -> p a"), 0.0)
            w4s = {}
            idxs = {}

            # offset loads for every image go first so each image's ph2 DVE
            # math can run while ph1 DMA traffic occupies the rings
            offms = {}
            for img in range(n_img):
                offm = constp.tile([128, 32, 18], f32, name=f"offm{img}")
                nc.sync.dma_start(
                    offm[:], off_d[img].rearrange("(s p) j -> p s j", s=32, p=128))
                offms[img] = offm

            # ---------- phase 2 (per image): index + weight math ----------
            def emit_ph2(img):
                with tc.tile_pool(name=f"sc{img}", bufs=1) as scp:
                    offm = offms[img]
                    offv = offm[:].rearrange("p s (t two) -> p s t two", two=2)

                    def t3(tag, dt=f32):
                        return scp.tile([128, 32, 9], dt, tag=tag, name=tag)

                    def fl(t):
                        # flat [128, 288] view: single contiguous inner dim
                        # (9-element inner runs cripple DVE throughput)
                        return t[:].rearrange("p a b -> p (a b)")

                    PY = t3("PY"); PX = t3("PX")
                    nc.vector.tensor_tensor(PY[:], offv[:, :, :, 0], byx[:, 0], Ao.add)
                    nc.vector.tensor_tensor(PX[:], offv[:, :, :, 1], byx[:, 1], Ao.add)
                    fy = t3("fy"); fx = t3("fx"); y0 = t3("y0"); x0 = t3("x0")
                    # floor via IEEE magic-number rint: floor(v)=rint(v-DELTA)
                    MAGIC, DELTA = 12582912.0, 0.49975586
                    for (dst, srcv) in ((y0, PY), (x0, PX)):
                        nc.vector.tensor_scalar(fl(dst), fl(srcv), -DELTA, None, Ao.add)
                        nc.vector.tensor_scalar(fl(dst), fl(dst), MAGIC, None, Ao.add)
                        nc.vector.tensor_scalar(fl(dst), fl(dst), -MAGIC, None, Ao.add)
                    nc.vector.tensor_tensor(fl(fy), fl(PY), fl(y0), Ao.subtract)
                    nc.vector.tensor_tensor(fl(fx), fl(PX), fl(x0), Ao.subtract)

                    wy0 = t3("wy0"); wy1 = fy; wx0 = t3("wx0"); wx1 = fx
                    nc.vector.tensor_scalar(fl(wy0), fl(fy), -1.0, 1.0, Ao.mult, Ao.add)
                    nc.vector.tensor_scalar(fl(wx0), fl(fx), -1.0, 1.0, Ao.mult, Ao.add)
                    ta = t3("ta"); tb = t3("tb")
                    wyv0 = t3("wyv0"); wyv1 = t3("wyv1")
                    wxv0 = t3("wxv0"); wxv1 = t3("wxv1")
                    for (src, wsrc, lo, hi, dst) in (
                            (y0, wy0, 0.0, 63.0, wyv0),
                            (y0, wy1, -1.0, 62.0, wyv1),
                            (x0, wx0, 0.0, 63.0, wxv0),
                            (x0, wx1, -1.0, 62.0, wxv1)):
                        nc.vector.tensor_scalar(fl(ta), fl(src), lo, None, Ao.is_ge)
                        nc.vector.tensor_scalar(fl(tb), fl(src), hi, None, Ao.is_le)
                        nc.vector.tensor_tensor(fl(ta), fl(ta), fl(tb), Ao.mult)
                        nc.vector.tensor_tensor(fl(dst), fl(wsrc), fl(ta), Ao.mult)
                    w00 = t3("w00"); w01 = t3("w01"); w10 = t3("w10"); w11 = t3("w11")
                    nc.vector.tensor_tensor(fl(w00), fl(wyv0), fl(wxv0), Ao.mult)
                    nc.vector.tensor_tensor(fl(w01), fl(wyv0), fl(wxv1), Ao.mult)
                    nc.vector.tensor_tensor(fl(w10), fl(wyv1), fl(wxv0), Ao.mult)
                    nc.vector.tensor_tensor(fl(w11), fl(wyv1), fl(wxv1), Ao.mult)

                    # w4[p, k, pb, sl, j, dup2]: each corner weight stored
                    # twice so the interp-mul broadcast AP ends in a packed
                    # [1,2] dim (enables the DVE 2x perf mode).
                    w4 = constp.tile([128, 9, NPB, SL, 4, 2], f16,
                                     name=f"w4_{img}")
                    w4s[img] = w4
                    w4v = w4[:].rearrange("p k b s j d -> p k b (s j d)")
                    for (wt, fo) in ((w00, 0), (w01, 1), (w10, 2), (w11, 3)):
                        for dup in range(2):
                            dst = bass.AP(w4.tensor, w4v.offset + fo * 2 + dup,
                                          [w4v.ap[0], [NPB * SL * 8, 9],
                                           [SL * 8, NPB], [8, SL]])
                            srcv = wt[:].rearrange("p (b s) k -> p k b s",
                                                   b=NPB, s=SL)
                            nc.vector.tensor_copy(dst, srcv)

                    # patch tile index:
                    # a = y0 mod 2, m = (y0-a)/2 clipped to [-1,32]; same for x
                    av = t3("av"); bv = t3("bv"); mv = t3("mv"); nv = t3("nv")
                    for (par, flv, srcv) in ((av, mv, y0), (bv, nv, x0)):
                        # fl = floor(src/2); par = src - 2*fl; fl clipped
                        nc.vector.tensor_scalar(fl(flv), fl(srcv), 0.5, -DELTA,
                                                Ao.mult, Ao.add)
                        nc.vector.tensor_scalar(fl(flv), fl(flv), MAGIC, None, Ao.add)
                        nc.vector.tensor_scalar(fl(flv), fl(flv), -MAGIC, None, Ao.add)
                        nc.vector.tensor_scalar(fl(par), fl(flv), -2.0, None, Ao.mult)
                        nc.vector.tensor_tensor(fl(par), fl(par), fl(srcv), Ao.add)
                        nc.vector.tensor_scalar(fl(flv), fl(flv), -1.0, 32.0,
                                                Ao.max, Ao.min)
                    # idx = (2a+b)*1156 + (m+1)*34 + (n+1)
                    iv = t3("iv"); tq = t3("tq")
                    nc.vector.tensor_scalar(fl(iv), fl(mv), 34.0, 35.0, Ao.mult, Ao.add)
                    nc.vector.tensor_tensor(fl(iv), fl(iv), fl(nv), Ao.add)
                    nc.vector.tensor_scalar(fl(tq), fl(av), float(2 * NT), None, Ao.mult)
                    nc.vector.tensor_tensor(fl(iv), fl(iv), fl(tq), Ao.add)
                    nc.vector.tensor_scalar(fl(tq), fl(bv), float(NT), None, Ao.mult)
                    nc.vector.tensor_tensor(fl(iv), fl(iv), fl(tq), Ao.add)
                    # staging: int16 tile indices in the natural [p, k, b, s]
                    # layout (p = pos%128)
                    idx_st = scp.tile([128, 9, NPB, SL], i16, tag="idx_st",
                                      name=f"idxst_{img}")
                    ivv = idx_st[:].rearrange("p k b s -> p k (b s)")
                    dst = bass.AP(idx_st.tensor, ivv.offset,
                                  [ivv.ap[0], [NPB * SL, 9], [SL, NPB], [1, SL]])
                    src = iv[:].rearrange("p (b s) k -> p k b s", b=NPB, s=SL)
                    nc.vector.tensor_copy(dst, src)

                    # dma_gather wants index i (= sl*128 + p) at
                    # [partition i%16, free i//16]: shuffle [g*16+q, k, b, s]
                    # -> [q, k, b, s*8+g] with 8 small SBUF->SBUF DMAs, then
                    # replicate to all 8 16-partition groups (each SWDGE Q7
                    # core pair reads the indices from its own group).
                    idx16 = constp.tile([128, 9, NPB, SL * 8], i16,
                                        name=f"idx16_{img}")
                    idxs[img] = idx16
                    i16v = idx16[0:16].rearrange(
                        "q k b (s g) -> q k b s g", g=8)
                    for g in range(8):
                        nc.scalar.dma_start(i16v[:, :, :, :, g],
                                            idx_st[g * 16:(g + 1) * 16])
                    for j in range(1, 8):
                        nc.scalar.dma_start(idx16[16 * j:16 * (j + 1)],
                                            idx16[0:16])

            E = 4 * C

            def emit_ph1(img, psum_bufs=1, head=True, exp=True):
                # ---------- phase 1: TC build ----------
                # zero only the pad slots of TC (the expansion DMAs cover
                # everything else). Grid rows/cols 0 and 33 are fully
                # pad; for odd parity the ragged half-rows/cols at 0/32 too.
                tcflat = TC_ds[img][:].rearrange("t e -> (t e)")
                if not head:
                    return emit_exp(img)

                def zdma(dst_off, dims, zsrc):
                    # HWDGE via ACT: keeps POOL free for the gather stream
                    nc.scalar.dma_start(
                        bass.AP(tcflat.tensor, tcflat.offset + dst_off, dims),
                        zsrc)

                def zrep(cnt):
                    # zpad rows replicated twice along a 0-stride free dim
                    return bass.AP(zpad.tensor, zpad[:].offset,
                                   [zpad[0:34, :].ap[0], [0, 2], [1, cnt]])

                def emit_zpads():
                    for a in range(2):
                        for b in range(2):
                            base = ((a * 2 + b) * NT) * E
                            # y pads (rows of the tile grid, all n)
                            if a == 0:
                                # rows 0 and 33 fully pad: one 3-dim op
                                zdma(base, [[1024, 34], [33 * G * E, 2],
                                            [1, 1024]],
                                     zrep(1024))
                            else:
                                zdma(base + 33 * G * E, [[1024, 34], [1, 1024]],
                                     zpad[0:34, :])
                                # row 0 r=0 and row 32 r=1 halves: one op
                                zdma(base, [[1024, 34], [32 * G * E + 512, 2],
                                            [1, 512]],
                                     zrep(512))
                            # x pads (cols of the tile grid, all m)
                            if b == 0:
                                zdma(base, [[G * E, 34], [33 * E, 2],
                                            [1, 1024]],
                                     zrep(1024))
                            else:
                                zdma(base + 33 * E, [[G * E, 34], [1, 1024]],
                                     zpad[0:34, :])
                                zdma(base + 0 * E,
                                     [[G * E, 34], [512, 2], [1, 256]],
                                     zpad[0:34, 0:512].rearrange(
                                         "p (u v) -> p u v", u=2))  # col0,s=0
                                zdma(base + 32 * E + 256,
                                     [[G * E, 34], [512, 2], [1, 256]],
                                     zpad[0:34, 0:512].rearrange(
                                         "p (u v) -> p u v", u=2))  # col32,s=1

                with tc.tile_pool(name=f"xp{img}", bufs=1) as xp, \
                     tc.tile_pool(name=f"xq{img}",
                                  bufs=2 if psum_bufs > 1 else 1) as xqp, \
                     tc.tile_pool(name=f"xps{img}", bufs=psum_bufs,
                                  space="PSUM") as xpp, \
                     tc.tile_pool(name=f"xst{img}", bufs=6) as xstp:
                    x16 = xp.tile([128, 2, HW], f16)
                    xv = x_d[img].rearrange("(cc p) q -> p cc q", cc=2, p=128)
                    for qc in range(4):
                        qs = slice(qc * 1024, (qc + 1) * 1024)
                        xq = xqp.tile([128, 2, 1024], f32, tag="xq")
                        nc.sync.dma_start(xq[:], xv[:, :, qs])
                        nc.scalar.activation(x16[:, :, qs], xq[:], Act.Copy)
                    # pads go after the x loads: they only have to land
                    # before the first gather, and issuing them first costs
                    # the critical x->cast->transpose chain ~30us of DMA slots
                    emit_zpads()
                    for qb in range(32):
                        st = xstp.tile([128, C], f16, tag="xst")
                        for cc in range(2):
                            ps = xpp.tile([128, 128], f16, tag="xps")
                            nc.tensor.transpose(
                                ps[:], x16[:, cc, qb * 128:(qb + 1) * 128], ident[:])
                            nc.scalar.activation(
                                st[:, cc * 128:(cc + 1) * 128], ps[:], Act.Copy)
                        nc.sync.dma_start(
                            xT_d[img, qb * 128:(qb + 1) * 128, :], st[:])
                        if exp and qb == 15:
                            # first expansion half only reads xT rows 0-33
                            # (stores 0-15); scalar ring, so no FIFO hazard
                            # with the remaining stores on the sync ring
                            emit_exp(img, halves=(0,), engines=(nc.scalar,))
                if exp:
                    emit_exp(img, halves=(1,))

            def emit_exp(img, halves=(0, 1), engines=None):
                # expansion: xT -> TC, 9 rectangular DRAM->DRAM DMAs per
                # parity copy, alternated across both HWDGE rings.
                if engines is None:
                    engines = (nc.sync, nc.scalar)
                xsrc = xT_d[img].rearrange("q c -> (q c)")
                tdst = TC_ds[img][:].rearrange("t e -> (t e)")
                ei = 0
                for a in range(2):
                    for b in range(2):
                        ab_base = ((a * 2 + b) * NT) * (4 * C)
                        # explicit (m0, mcnt, y_base, r) blocks; y = row of r-th
                        # patch pixel. a=0: y=2m+r. a=1: y=2m+1+r.
                        yparts = ([(0, 32, 0, 0), (0, 32, 1, 1)] if a == 0
                                  else [(-1, 32, 0, 1), (0, 32, 1, 0)])
                        # x blocks; b=0 keeps s merged into the inner dim.
                        xparts = ([(0, 32, 0, None)] if b == 0
                                  else [(-1, 32, 0, 1), (0, 32, 1, 0)])
                        yhalves = [(m0 + h * 16, 16, y_base + h * 32, r)
                                   for (m0, mcnt, y_base, r) in yparts
                                   for h in range(2) if h in halves]
                        for (m0, mcnt, y_base, r) in yhalves:
                            for (n0, ncnt, x_base, s) in xparts:
                                s0 = 0 if s is None else s
                                inner = 2 * C if s is None else C
                                src_off = (y_base * W + x_base) * C
                                dst_off = (ab_base
                                           + ((m0 + 1) * G + (n0 + 1)) * 4 * C
                                           + (r * 2 + s0) * C)
                                sdims = [[2 * W * C, mcnt], [2 * C, ncnt],
                                         [1, inner]]
                                ddims = [[G * 4 * C, mcnt], [4 * C, ncnt],
                                         [1, inner]]
                                src_ap = bass.AP(xsrc.tensor,
                                                 xsrc.offset + src_off, sdims)
                                dst_ap = bass.AP(tdst.tensor,
                                                 tdst.offset + dst_off, ddims)
                                engines[ei % len(engines)].dma_start(
                                    dst_ap, src_ap)
                                ei += 1

            # ---------- phase 3: taps (ph1 of img+1 interleaved) ----------
            emit_ph2(0)
            emit_ph1(0, psum_bufs=4)
            for img in range(1, n_img):
                emit_ph2(img)
            with tc.tile_pool(name="gp", bufs=4) as gpp, \
                 tc.tile_pool(name="zp", bufs=2) as zpp, \
                 tc.tile_pool(name="ztp", bufs=2) as ztp, \
                 tc.tile_pool(name="fp", bufs=2) as fpp, \
                 tc.tile_pool(name="pp", bufs=2, space="PSUM") as psp, \
                 tc.tile_pool(name="ac", bufs=1, space="PSUM") as accp, \
                 tc.tile_pool(name="cp", bufs=1, space="PSUM") as clsp:
                def emit_ph3(img, pbs):
                    w4 = w4s[img]
                    idx16 = idxs[img]
                    for pb in pbs:
                        acc = [accp.tile([128, 2, 512], f32, tag=f"acc{oc}", name=f"acc{oc}")
                               for oc in range(2)]
                        for k in range(9):
                            patch = gpp.tile([128, SL, 4 * C], f16, tag="patch")
                            nc.gpsimd.dma_gather(
                                out_ap=patch[:],
                                in_ap=TC_ds[img][:],
                                idxs_ap=idx16[:, k, pb, :],
                                num_idxs=PB,
                                num_idxs_reg=PB,
                                elem_size=4 * C,
                                queue_num=(img * 36 + pb * 9 + k) % 4,
                            )
                            m = zpp.tile([128, SL, 4, C], f16, tag="m")
                            wsl = w4[:, k, pb]  # [128, SL, 4, 2]
                            wap = bass.AP(
                                wsl.tensor, wsl.offset,
                                [wsl.ap[0], [2, SL * 4], [0, C // 2], [1, 2]])
                            nc.vector.tensor_tensor(
                                m[:].rearrange("p s j (ch d) -> p (s j) ch d",
                                               d=2),
                                patch[:].rearrange(
                                    "p s (j ch d) -> p (s j) ch d",
                                    j=4, d=2),
                                wap, Ao.mult)
                            # HW does NOT accumulate transpose matmuls in
                            # PSUM (CoreSim models it, silicon disagrees), so
                            # the corner sum stays on the DVE
                            a12 = zpp.tile([128, SL, 2, C], f16, tag="a12")
                            z = zpp.tile([128, SL, C], f16, tag="z")
                            nc.vector.tensor_tensor(
                                a12[:], m[:, :, 0:2, :], m[:, :, 2:4, :], Ao.add)
                            nc.vector.tensor_tensor(
                                z[:], a12[:, :, 0, :], a12[:, :, 1, :], Ao.add)

                            zT = ztp.tile([128, 2, PB], f16, tag="zT")
                            for cc in range(2):
                                for g in range(2):
                                    ps = psp.tile([128, 4, 128], f16, tag="pst")
                                    for j in range(4):
                                        sl = g * 4 + j
                                        nc.tensor.transpose(
                                            ps[:, j],
                                            z[:, sl, cc * 128:(cc + 1) * 128],
                                            ident[:])
                                    nc.scalar.activation(
                                        zT[:, cc, g * 512:(g + 1) * 512],
                                        ps[:].rearrange("p a b -> p (a b)"),
                                        Act.Copy)
                            for cc in range(2):
                                for oc in range(2):
                                    for nb in range(2):
                                        nc.tensor.matmul(
                                            acc[oc][:, nb],
                                            wdef[:, k, cc, oc],
                                            zT[:, cc, nb * 512:(nb + 1) * 512],
                                            start=(k == 0 and cc == 0),
                                            stop=(k == 8 and cc == 1))
                        feat = fpp.tile([128, 2, PB], f16, tag="feat")
                        for oc in range(2):
                            nc.scalar.activation(
                                feat[:, oc],
                                acc[oc][:].rearrange("p a b -> p (a b)"),
                                Act.Relu)
                        for half in range(2):
                            cps = clsp.tile([1, 512], f32, tag="cls")
                            for oc in range(2):
                                nc.tensor.matmul(
                                    cps[:], wcls[:, oc:oc + 1],
                                    feat[:, oc, half * 512:(half + 1) * 512],
                                    start=(oc == 0), stop=(oc == 1))
                            co = fpp.tile([1, 512], f32, tag="co")
                            nc.vector.tensor_tensor(
                                co[:], cps[:],
                                bcls[:].to_broadcast([1, 512]), Ao.add)
                            nc.scalar.dma_start(
                                out_d[img,
                                      pb * PB + half * 512:
                                      pb * PB + (half + 1) * 512]
                                .rearrange("a -> () a"),
                                co[:])

                for img in range(n_img):
                    emit_ph3(img, [0])
                    if img + 1 < n_img:
                        # the whole next-image TC build (x load, transposes,
                        # 17.5MB expansion) hides under this image's taps,
                        # keeping the head free for image 0's build alone
                        emit_ph1(img + 1, head=True, exp=False)
                        emit_ph3(img, [1])
                        emit_ph1(img + 1, head=False)
                        emit_ph3(img, range(2, NPB))
                    else:
                        emit_ph3(img, range(1, NPB))
    nc.compile()
    return nc


def make_in_map(x_img, off_img, w_def, w_cls, b_cls, ident, byx, wp, wcp):
    n_img = x_img.shape[0]
    return {
        "x": np.ascontiguousarray(x_img.reshape(n_img, C, HW).astype(np.float32)),
        "off": np.ascontiguousarray(off_img.astype(np.float32)),
        "wdef": wp,
        "wcls": wcp,
        "bcls": np.asarray(b_cls, dtype=np.float32).reshape(1, 1),
        "ident": ident,
        "byx": byx,
    }


_CACHE = {}


def _get_nc(n_img):
    if n_img not in _CACHE:
        _CACHE[n_img] = build(n_img)
    return _CACHE[n_img]


def kernel(x, offset, w_def, w_cls, b_cls):
    x = np.asarray(x, dtype=np.float32)
    offset = np.asarray(offset, dtype=np.float32)
    w_def = np.asarray(w_def, dtype=np.float32)
    w_cls = np.asarray(w_cls, dtype=np.float32)
    b_cls = np.asarray(b_cls, dtype=np.float32)
    N = x.shape[0]
    n_img = (N + N_CORES - 1) // N_CORES
    assert n_img * N_CORES == N, "batch must split evenly across 8 cores"

    ident, byx = host_constants()
    wp, wcp = pack_weights(w_def, w_cls)
    nc = _get_nc(n_img)

    in_maps = []
    for cix in range(N_CORES):
        sl = slice(cix * n_img, (cix + 1) * n_img)
        in_maps.append(make_in_map(
            x[sl].reshape(n_img, C, HW), offset[sl],
            w_def, w_cls, b_cls, ident, byx, wp, wcp))

    from concourse.bass_utils import run_bass_kernel_spmd
    res = run_bass_kernel_spmd(nc, in_maps, list(range(N_CORES)))
    outs = [res.results[cix]["out"].reshape(n_img, 1, H, W)
            for cix in range(N_CORES)]
    return np.concatenate(outs, axis=0).astype(np.float32)



# revision 57
# speedup vs baseline: 1.0979x; 1.0979x over previous
"""TRN2 Bass kernel for nn_AlignHead (deformable conv 3x3 + ReLU + 1x1 cls).

Strategy: pure data parallel over batch (2 images per NeuronCore, 8 cores).
On-device pipeline per image:
  1. offsets -> patch tile indices + bilinear corner weights on DVE
     (floor via the IEEE magic-number rint trick; out-of-bounds corners get
     zero weight, matching the reference's clamp+mask semantics).
  2. cast x to f16, PE-transpose to xT [HW, C], then build 4 parity-aligned
     2x2-tiled copies of the image (TC): for any integer corner (y0, x0) the
     bilinear 2x2 patch is ONE contiguous 2KB element in copy (y0&1, x0&1),
     so one gather descriptor fetches all 4 corners.
  3. per (tap, 1024-position block): ONE bulk dma_gather (1024 int16
     indices, replicated into all 8 16-partition groups for the Q7 core
     pairs; ~1.3us of Pool DGE time vs ~8us for 8 indirect_dma_starts),
     DVE bilinear interp (1 broadcast mul in the DVE 2x perf mode + 2
     paired adds), PE transpose back to channel-major, PE conv
     accumulation over the 9 taps in PSUM.
  4. fused ReLU eviction, 1x1 cls matmul on PE, bias add, DMA out.

Scheduling: image 0's TC build forms the pipeline head (expansion DMAs
split across both HWDGE rings, first y-half issued after 16 of the 32
xT stores); image i+1's x load/transpose runs inside image i's head and
its 17.5MB expansion hides under image i's tap compute. The steady state
is jointly DMA- (~550us of SDMA busy) and DVE-bound (~590us interp).

Known HW/CoreSim divergence: PSUM accumulation across PE *transpose*
matmuls (start/stop over is_transpose=True) works in CoreSim but NOT on
silicon - the bilinear corner sum must stay on the DVE.

All gather/matmul data is fp16 with fp32 PSUM accumulation; overall
relative error vs the fp32 reference is ~7e-4.
"""

import sys
sys.path.insert(0, '/opt/trn_rl_repo')
import numpy as np

import concourse.bass as bass
import concourse.tile as tile
from concourse import bacc, mybir

f32 = mybir.dt.float32
f16 = mybir.dt.float16
i32 = mybir.dt.int32
i16 = mybir.dt.int16

N_CORES = 8
C = 256
H = W = 64
HW = H * W
NPB = 4             # position blocks per image
PB = HW // NPB      # 1024 positions per block
SL = PB // 128      # 8 slots (gathers) per block
G = 34              # tile grid per axis (m in [-1,32] stored at m+1)
NT = G * G          # tiles per parity copy
TCN = 4 * NT        # tiles per image (4 parity copies)
Ao = mybir.AluOpType
Act = mybir.ActivationFunctionType


def host_constants():
    ident = np.eye(128, dtype=np.float16)
    p = np.arange(128)
    s = np.arange(32)
    k = np.arange(9)
    pos = s[None, :, None] * 128 + p[:, None, None]
    y = pos // W
    x = pos % W
    ky = k // 3
    kx = k % 3
    byx = np.empty((128, 2, 32, 9), dtype=np.float32)
    byx[:, 0] = y + (ky[None, None, :] - 1)
    byx[:, 1] = x + (kx[None, None, :] - 1)
    return ident, byx


def pack_weights(w_def, w_cls):
    w = np.asarray(w_def, dtype=np.float32).reshape(256, 256, 9)
    wp = np.empty((128, 9, 2, 2, 128), dtype=np.float32)
    for cc in range(2):
        for oc in range(2):
            blk = w[oc * 128:(oc + 1) * 128, cc * 128:(cc + 1) * 128, :]
            wp[:, :, cc, oc, :] = blk.transpose(1, 2, 0)
    wc = np.asarray(w_cls, dtype=np.float32).reshape(256)
    wcp = wc.reshape(2, 128).T.copy()
    return wp, wcp


def build(n_img):
    nc = bacc.Bacc("TRN2", target_bir_lowering=False, debug=False,
                   num_devices=N_CORES, num_swdge_queues=4)
    x_d = nc.dram_tensor("x", [n_img, C, HW], f32, kind="ExternalInput").ap()
    off_d = nc.dram_tensor("off", [n_img, HW, 18], f32, kind="ExternalInput").ap()
    wdef_d = nc.dram_tensor("wdef", [128, 9, 2, 2, 128], f32, kind="ExternalInput").ap()
    wcls_d = nc.dram_tensor("wcls", [128, 2], f32, kind="ExternalInput").ap()
    bcls_d = nc.dram_tensor("bcls", [1, 1], f32, kind="ExternalInput").ap()
    ident_d = nc.dram_tensor("ident", [128, 128], f16, kind="ExternalInput").ap()
    byx_d = nc.dram_tensor("byx", [128, 2, 32, 9], f32, kind="ExternalInput").ap()
    out_d = nc.dram_tensor("out", [n_img, HW], f32, kind="ExternalOutput").ap()

    xT_d = nc.dram_tensor("xT", [n_img, HW, C], f16).ap()
    TC_ds = [nc.dram_tensor(f"TC{i}", [TCN, 4 * C], f16).ap()
             for i in range(n_img)]

    with tile.TileContext(nc) as tc:
        with tc.tile_pool(name="const", bufs=1) as constp:
            ident = constp.tile([128, 128], f16)
            nc.sync.dma_start(ident[:], ident_d[:])
            byx = constp.tile([128, 2, 32, 9], f32)
            nc.sync.dma_start(byx[:], byx_d[:])
            wdef = constp.tile([128, 9, 2, 2, 128], f16)
            with tc.tile_pool(name="wstage", bufs=1) as wsp:
                wst = wsp.tile([128, 9, 2, 2, 128], f32)
                nc.sync.dma_start(wst[:], wdef_d[:])
                nc.scalar.activation(wdef[:], wst[:], Act.Copy)
            wcls = constp.tile([128, 2], f16)
            wclsf = constp.tile([128, 2], f32)
            nc.sync.dma_start(wclsf[:], wcls_d[:])
            nc.vector.tensor_copy(wcls[:], wclsf[:])
            bcls = constp.tile([1, 1], f32)
            nc.sync.dma_start(bcls[:], bcls_d[:])
            zpad = constp.tile([128, 1024], f16)
            nc.vector.memset(zpad[:].rearrange("p a -> p a"), 0.0)
            w4s = {}
            idxs = {}

            # offset loads for every image go first so each image's ph2 DVE
            # math can run while ph1 DMA traffic occupies the rings
            offms = {}
            for img in range(n_img):
                offm = constp.tile([128, 32, 18], f32, name=f"offm{img}")
                nc.sync.dma_start(
                    offm[:], off_d[img].rearrange("(s p) j -> p s j", s=32, p=128))
                offms[img] = offm

            # ---------- phase 2 (per image): index + weight math ----------
            def emit_ph2(img):
                with tc.tile_pool(name=f"sc{img}", bufs=1) as scp:
                    offm = offms[img]
                    offv = offm[:].rearrange("p s (t two) -> p s t two", two=2)

                    def t3(tag, dt=f32):
                        return scp.tile([128, 32, 9], dt, tag=tag, name=tag)

                    def fl(t):
                        # flat [128, 288] view: single contiguous inner dim
                        # (9-element inner runs cripple DVE throughput)
                        return t[:].rearrange("p a b -> p (a b)")

                    PY = t3("PY"); PX = t3("PX")
                    nc.vector.tensor_tensor(PY[:], offv[:, :, :, 0], byx[:, 0], Ao.add)
                    nc.vector.tensor_tensor(PX[:], offv[:, :, :, 1], byx[:, 1], Ao.add)
                    fy = t3("fy"); fx = t3("fx"); y0 = t3("y0"); x0 = t3("x0")
                    # floor via IEEE magic-number rint: floor(v)=rint(v-DELTA)
                    MAGIC, DELTA = 12582912.0, 0.49975586
                    for (dst, srcv) in ((y0, PY), (x0, PX)):
                        nc.vector.tensor_scalar(fl(dst), fl(srcv), -DELTA, None, Ao.add)
                        nc.vector.tensor_scalar(fl(dst), fl(dst), MAGIC, None, Ao.add)
                        nc.vector.tensor_scalar(fl(dst), fl(dst), -MAGIC, None, Ao.add)
                    nc.vector.tensor_tensor(fl(fy), fl(PY), fl(y0), Ao.subtract)
                    nc.vector.tensor_tensor(fl(fx), fl(PX), fl(x0), Ao.subtract)

                    wy0 = t3("wy0"); wy1 = fy; wx0 = t3("wx0"); wx1 = fx
                    nc.vector.tensor_scalar(fl(wy0), fl(fy), -1.0, 1.0, Ao.mult, Ao.add)
                    nc.vector.tensor_scalar(fl(wx0), fl(fx), -1.0, 1.0, Ao.mult, Ao.add)
                    ta = t3("ta"); tb = t3("tb")
                    wyv0 = t3("wyv0"); wyv1 = t3("wyv1")
                    wxv0 = t3("wxv0"); wxv1 = t3("wxv1")
                    for (src, wsrc, lo, hi, dst) in (
                            (y0, wy0, 0.0, 63.0, wyv0),
                            (y0, wy1, -1.0, 62.0, wyv1),
                            (x0, wx0, 0.0, 63.0, wxv0),
                            (x0, wx1, -1.0, 62.0, wxv1)):
                        nc.vector.tensor_scalar(fl(ta), fl(src), lo, None, Ao.is_ge)
                        nc.vector.tensor_scalar(fl(tb), fl(src), hi, None, Ao.is_le)
                        nc.vector.tensor_tensor(fl(ta), fl(ta), fl(tb), Ao.mult)
                        nc.vector.tensor_tensor(fl(dst), fl(wsrc), fl(ta), Ao.mult)
                    w00 = t3("w00"); w01 = t3("w01"); w10 = t3("w10"); w11 = t3("w11")
                    nc.vector.tensor_tensor(fl(w00), fl(wyv0), fl(wxv0), Ao.mult)
                    nc.vector.tensor_tensor(fl(w01), fl(wyv0), fl(wxv1), Ao.mult)
                    nc.vector.tensor_tensor(fl(w10), fl(wyv1), fl(wxv0), Ao.mult)
                    nc.vector.tensor_tensor(fl(w11), fl(wyv1), fl(wxv1), Ao.mult)

                    # w4[p, k, pb, sl, j, dup2]: each corner weight stored
                    # twice so the interp-mul broadcast AP ends in a packed
                    # [1,2] dim (enables the DVE 2x perf mode).
                    w4 = constp.tile([128, 9, NPB, SL, 4, 2], f16,
                                     name=f"w4_{img}")
                    w4s[img] = w4
                    w4v = w4[:].rearrange("p k b s j d -> p k b (s j d)")
                    for (wt, fo) in ((w00, 0), (w01, 1), (w10, 2), (w11, 3)):
                        for dup in range(2):
                            dst = bass.AP(w4.tensor, w4v.offset + fo * 2 + dup,
                                          [w4v.ap[0], [NPB * SL * 8, 9],
                                           [SL * 8, NPB], [8, SL]])
                            srcv = wt[:].rearrange("p (b s) k -> p k b s",
                                                   b=NPB, s=SL)
                            nc.vector.tensor_copy(dst, srcv)

                    # patch tile index:
                    # a = y0 mod 2, m = (y0-a)/2 clipped to [-1,32]; same for x
                    av = t3("av"); bv = t3("bv"); mv = t3("mv"); nv = t3("nv")
                    for (par, flv, srcv) in ((av, mv, y0), (bv, nv, x0)):
                        # fl = floor(src/2); par = src - 2*fl; fl clipped
                        nc.vector.tensor_scalar(fl(flv), fl(srcv), 0.5, -DELTA,
                                                Ao.mult, Ao.add)
                        nc.vector.tensor_scalar(fl(flv), fl(flv), MAGIC, None, Ao.add)
                        nc.vector.tensor_scalar(fl(flv), fl(flv), -MAGIC, None, Ao.add)
                        nc.vector.tensor_scalar(fl(par), fl(flv), -2.0, None, Ao.mult)
                        nc.vector.tensor_tensor(fl(par), fl(par), fl(srcv), Ao.add)
                        nc.vector.tensor_scalar(fl(flv), fl(flv), -1.0, 32.0,
                                                Ao.max, Ao.min)
                    # idx = (2a+b)*1156 + (m+1)*34 + (n+1)
                    iv = t3("iv"); tq = t3("tq")
                    nc.vector.tensor_scalar(fl(iv), fl(mv), 34.0, 35.0, Ao.mult, Ao.add)
                    nc.vector.tensor_tensor(fl(iv), fl(iv), fl(nv), Ao.add)
                    nc.vector.tensor_scalar(fl(tq), fl(av), float(2 * NT), None, Ao.mult)
                    nc.vector.tensor_tensor(fl(iv), fl(iv), fl(tq), Ao.add)
                    nc.vector.tensor_scalar(fl(tq), fl(bv), float(NT), None, Ao.mult)
                    nc.vector.tensor_tensor(fl(iv), fl(iv), fl(tq), Ao.add)
                    # staging: int16 tile indices in the natural [p, k, b, s]
                    # layout (p = pos%128)
                    idx_st = scp.tile([128, 9, NPB, SL], i16, tag="idx_st",
                                      name=f"idxst_{img}")
                    ivv = idx_st[:].rearrange("p k b s -> p k (b s)")
                    dst = bass.AP(idx_st.tensor, ivv.offset,
                                  [ivv.ap[0], [NPB * SL, 9], [SL, NPB], [1, SL]])
                    src = iv[:].rearrange("p (b s) k -> p k b s", b=NPB, s=SL)
                    nc.vector.tensor_copy(dst, src)

                    # dma_gather wants index i (= sl*128 + p) at
                    # [partition i%16, free i//16]: shuffle [g*16+q, k, b, s]
                    # -> [q, k, b, s*8+g] with 8 small SBUF->SBUF DMAs, then
                    # replicate to all 8 16-partition groups (each SWDGE Q7
                    # core pair reads the indices from its own group).
                    idx16 = constp.tile([128, 9, NPB, SL * 8], i16,
                                        name=f"idx16_{img}")
                    idxs[img] = idx16
                    i16v = idx16[0:16].rearrange(
                        "q k b (s g) -> q k b s g", g=8)
                    for g in range(8):
                        nc.scalar.dma_start(i16v[:, :, :, :, g],
                                            idx_st[g * 16:(g + 1) * 16])
                    for j in range(1, 8):
                        nc.scalar.dma_start(idx16[16 * j:16 * (j + 1)],
                                            idx16[0:16])

            E = 4 * C

            def emit_ph1(img, psum_bufs=1, head=True, exp=True):
                # ---------- phase 1: TC build ----------
                # zero only the pad slots of TC (the expansion DMAs cover
                # everything else). Grid rows/cols 0 and 33 are fully
                # pad; for odd parity the ragged half-rows/cols at 0/32 too.
                tcflat = TC_ds[img][:].rearrange("t e -> (t e)")
                if not head:
                    return emit_exp(img)

                def zdma(dst_off, dims, zsrc):
                    # HWDGE via ACT: keeps POOL free for the gather stream
                    nc.scalar.dma_start(
                        bass.AP(tcflat.tensor, tcflat.offset + dst_off, dims),
                        zsrc)

                def zrep(cnt):
                    # zpad rows replicated twice along a 0-stride free dim
                    return bass.AP(zpad.tensor, zpad[:].offset,
                                   [zpad[0:34, :].ap[0], [0, 2], [1, cnt]])

                def emit_zpads():
                    for a in range(2):
                        for b in range(2):
                            base = ((a * 2 + b) * NT) * E
                            # y pads (rows of the tile grid, all n)
                            if a == 0:
                                # rows 0 and 33 fully pad: one 3-dim op
                                zdma(base, [[1024, 34], [33 * G * E, 2],
                                            [1, 1024]],
                                     zrep(1024))
                            else:
                                zdma(base + 33 * G * E, [[1024, 34], [1, 1024]],
                                     zpad[0:34, :])
                                # row 0 r=0 and row 32 r=1 halves: one op
                                zdma(base, [[1024, 34], [32 * G * E + 512, 2],
                                            [1, 512]],
                                     zrep(512))
                            # x pads (cols of the tile grid, all m)
                            if b == 0:
                                zdma(base, [[G * E, 34], [33 * E, 2],
                                            [1, 1024]],
                                     zrep(1024))
                            else:
                                zdma(base + 33 * E, [[G * E, 34], [1, 1024]],
                                     zpad[0:34, :])
                                zdma(base + 0 * E,
                                     [[G * E, 34], [512, 2], [1, 256]],
                                     zpad[0:34, 0:512].rearrange(
                                         "p (u v) -> p u v", u=2))  # col0,s=0
                                zdma(base + 32 * E + 256,
                                     [[G * E, 34], [512, 2], [1, 256]],
                                     zpad[0:34, 0:512].rearrange(
                                         "p (u v) -> p u v", u=2))  # col32,s=1

                with tc.tile_pool(name=f"xp{img}", bufs=1) as xp, \
                     tc.tile_pool(name=f"xq{img}",
                                  bufs=2 if psum_bufs > 1 else 1) as xqp, \
                     tc.tile_pool(name=f"xps{img}", bufs=psum_bufs,
                                  space="PSUM") as xpp, \
                     tc.tile_pool(name=f"xst{img}", bufs=6) as xstp:
                    x16 = xp.tile([128, 2, HW], f16)
                    xv = x_d[img].rearrange("(cc p) q -> p cc q", cc=2, p=128)
                    for qc in range(4):
                        qs = slice(qc * 1024, (qc + 1) * 1024)
                        xq = xqp.tile([128, 2, 1024], f32, tag="xq")
                        nc.sync.dma_start(xq[:], xv[:, :, qs])
                        nc.scalar.activation(x16[:, :, qs], xq[:], Act.Copy)
                    # pads go after the x loads: they only have to land
                    # before the first gather, and issuing them first costs
                    # the critical x->cast->transpose chain ~30us of DMA slots
                    emit_zpads()
                    for qb in range(32):
                        st = xstp.tile([128, C], f16, tag="xst")
                        for cc in range(2):
                            ps = xpp.tile([128, 128], f16, tag="xps")
                            nc.tensor.transpose(
                                ps[:], x16[:, cc, qb * 128:(qb + 1) * 128], ident[:])
                            nc.scalar.activation(
                                st[:, cc * 128:(cc + 1) * 128], ps[:], Act.Copy)
                        nc.sync.dma_start(
                            xT_d[img, qb * 128:(qb + 1) * 128, :], st[:])
                        if exp and qb == 15:
                            # first expansion half only reads xT rows 0-33
                            # (stores 0-15); scalar ring, so no FIFO hazard
                            # with the remaining stores on the sync ring
                            emit_exp(img, halves=(0,), engines=(nc.scalar,))
                if exp:
                    emit_exp(img, halves=(1,))

            def emit_exp(img, halves=(0, 1), engines=None):
                # expansion: xT -> TC, 9 rectangular DRAM->DRAM DMAs per
                # parity copy, alternated across both HWDGE rings.
                if engines is None:
                    engines = (nc.sync, nc.scalar)
                xsrc = xT_d[img].rearrange("q c -> (q c)")
                tdst = TC_ds[img][:].rearrange("t e -> (t e)")
                ei = 0
                for a in range(2):
                    for b in range(2):
                        ab_base = ((a * 2 + b) * NT) * (4 * C)
                        # explicit (m0, mcnt, y_base, r) blocks; y = row of r-th
                        # patch pixel. a=0: y=2m+r. a=1: y=2m+1+r.
                        yparts = ([(0, 32, 0, 0), (0, 32, 1, 1)] if a == 0
                                  else [(-1, 32, 0, 1), (0, 32, 1, 0)])
                        # x blocks; b=0 keeps s merged into the inner dim.
                        xparts = ([(0, 32, 0, None)] if b == 0
                                  else [(-1, 32, 0, 1), (0, 32, 1, 0)])
                        yhalves = [(m0 + h * 16, 16, y_base + h * 32, r)
                                   for (m0, mcnt, y_base, r) in yparts
                                   for h in range(2) if h in halves]
                        for (m0, mcnt, y_base, r) in yhalves:
                            for (n0, ncnt, x_base, s) in xparts:
                                s0 = 0 if s is None else s
                                inner = 2 * C if s is None else C
                                src_off = (y_base * W + x_base) * C
                                dst_off = (ab_base
                                           + ((m0 + 1) * G + (n0 + 1)) * 4 * C
                                           + (r * 2 + s0) * C)
                                sdims = [[2 * W * C, mcnt], [2 * C, ncnt],
                                         [1, inner]]
                                ddims = [[G * 4 * C, mcnt], [4 * C, ncnt],
                                         [1, inner]]
                                src_ap = bass.AP(xsrc.tensor,
                                                 xsrc.offset + src_off, sdims)
                                dst_ap = bass.AP(tdst.tensor,
                                                 tdst.offset + dst_off, ddims)
                                engines[ei % len(engines)].dma_start(
                                    dst_ap, src_ap)
                                ei += 1

            # ---------- phase 3: taps (ph1 of img+1 interleaved) ----------
            emit_ph2(0)
            emit_ph1(0, psum_bufs=4)
            for img in range(1, n_img):
                emit_ph2(img)
            with tc.tile_pool(name="gp", bufs=4) as gpp, \
                 tc.tile_pool(name="zp", bufs=2) as zpp, \
                 tc.tile_pool(name="ztp", bufs=2) as ztp, \
                 tc.tile_pool(name="fp", bufs=2) as fpp, \
                 tc.tile_pool(name="pp", bufs=2, space="PSUM") as psp, \
                 tc.tile_pool(name="ac", bufs=1, space="PSUM") as accp, \
                 tc.tile_pool(name="cp", bufs=1, space="PSUM") as clsp:
                def emit_ph3(img, pbs):
                    w4 = w4s[img]
                    idx16 = idxs[img]
                    for pb in pbs:
                        acc = [accp.tile([128, 2, 512], f32, tag=f"acc{oc}", name=f"acc{oc}")
                               for oc in range(2)]
                        for k in range(9):
                            patch = gpp.tile([128, SL, 4 * C], f16, tag="patch")
                            nc.gpsimd.dma_gather(
                                out_ap=patch[:],
                                in_ap=TC_ds[img][:],
                                idxs_ap=idx16[:, k, pb, :],
                                num_idxs=PB,
                                num_idxs_reg=PB,
                                elem_size=4 * C,
                                queue_num=(img * 36 + pb * 9 + k) % 4,
                            )
                            m = zpp.tile([128, SL, 4, C], f16, tag="m")
                            wsl = w4[:, k, pb]  # [128, SL, 4, 2]
                            wap = bass.AP(
                                wsl.tensor, wsl.offset,
                                [wsl.ap[0], [2, SL * 4], [0, C // 2], [1, 2]])
                            nc.vector.tensor_tensor(
                                m[:].rearrange("p s j (ch d) -> p (s j) ch d",
                                               d=2),
                                patch[:].rearrange(
                                    "p s (j ch d) -> p (s j) ch d",
                                    j=4, d=2),
                                wap, Ao.mult)
                            # HW does NOT accumulate transpose matmuls in
                            # PSUM (CoreSim models it, silicon disagrees), so
                            # the corner sum stays on the DVE
                            a12 = zpp.tile([128, SL, 2, C], f16, tag="a12")
                            z = zpp.tile([128, SL, C], f16, tag="z")
                            nc.vector.tensor_tensor(
                                a12[:], m[:, :, 0:2, :], m[:, :, 2:4, :], Ao.add)
                            nc.vector.tensor_tensor(
                                z[:], a12[:, :, 0, :], a12[:, :, 1, :], Ao.add)

                            zT = ztp.tile([128, 2, PB], f16, tag="zT")
                            for cc in range(2):
                                for g in range(2):
                                    ps = psp.tile([128, 4, 128], f16, tag="pst")
                                    for j in range(4):
                                        sl = g * 4 + j
                                        nc.tensor.transpose(
                                            ps[:, j],
                                            z[:, sl, cc * 128:(cc + 1) * 128],
                                            ident[:])
                                    nc.scalar.activation(
                                        zT[:, cc, g * 512:(g + 1) * 512],
                                        ps[:].rearrange("p a b -> p (a b)"),
                                        Act.Copy)
                            for cc in range(2):
                                for oc in range(2):
                                    for nb in range(2):
                                        nc.tensor.matmul(
                                            acc[oc][:, nb],
                                            wdef[:, k, cc, oc],
                                            zT[:, cc, nb * 512:(nb + 1) * 512],
                                            start=(k == 0 and cc == 0),
                                            stop=(k == 8 and cc == 1))
                        feat = fpp.tile([128, 2, PB], f16, tag="feat")
                        for oc in range(2):
                            nc.scalar.activation(
                                feat[:, oc],
                                acc[oc][:].rearrange("p a b -> p (a b)"),
                                Act.Relu)
                        for half in range(2):
                            cps = clsp.tile([1, 512], f32, tag="cls")
                            for oc in range(2):
                                nc.tensor.matmul(
                                    cps[:], wcls[:, oc:oc + 1],
                                    feat[:, oc, half * 512:(half + 1) * 512],
                                    start=(oc == 0), stop=(oc == 1))
                            co = fpp.tile([1, 512], f32, tag="co")
                            nc.vector.tensor_tensor(
                                co[:], cps[:],
                                bcls[:].to_broadcast([1, 512]), Ao.add)
                            nc.scalar.dma_start(
                                out_d[img,
                                      pb * PB + half * 512:
                                      pb * PB + (half + 1) * 512]
                                .rearrange("a -> () a"),
                                co[:])

                for img in range(n_img):
                    emit_ph3(img, [0])
                    if img + 1 < n_img:
                        # the whole next-image TC build (x load, transposes,
                        # 17.5MB expansion) hides under this image's taps,
                        # keeping the head free for image 0's build alone
                        emit_ph1(img + 1, head=True, exp=False)
                        emit_ph3(img, [1])
                        emit_ph1(img + 1, head=False)
                        emit_ph3(img, range(2, NPB))
                    else:
                        emit_ph3(img, range(1, NPB))
    nc.compile()
    return nc


def make_in_map(x_img, off_img, w_def, w_cls, b_cls, ident, byx, wp, wcp):
    n_img = x_img.shape[0]
    return {
        "x": np.ascontiguousarray(x_img.reshape(n_img, C, HW).astype(np.float32)),
        "off": np.ascontiguousarray(off_img.astype(np.float32)),
        "wdef": wp,
        "wcls": wcp,
        "bcls": np.asarray(b_cls, dtype=np.float32).reshape(1, 1),
        "ident": ident,
        "byx": byx,
    }


_CACHE = {}


def _get_nc(n_img):
    if n_img not in _CACHE:
        _CACHE[n_img] = build(n_img)
    return _CACHE[n_img]


def kernel(x, offset, w_def, w_cls, b_cls):
    x = np.asarray(x, dtype=np.float32)
    offset = np.asarray(offset, dtype=np.float32)
    w_def = np.asarray(w_def, dtype=np.float32)
    w_cls = np.asarray(w_cls, dtype=np.float32)
    b_cls = np.asarray(b_cls, dtype=np.float32)
    N = x.shape[0]
    n_img = (N + N_CORES - 1) // N_CORES
    assert n_img * N_CORES == N, "batch must split evenly across 8 cores"

    ident, byx = host_constants()
    wp, wcp = pack_weights(w_def, w_cls)
    nc = _get_nc(n_img)

    in_maps = []
    for cix in range(N_CORES):
        sl = slice(cix * n_img, (cix + 1) * n_img)
        in_maps.append(make_in_map(
            x[sl].reshape(n_img, C, HW), offset[sl],
            w_def, w_cls, b_cls, ident, byx, wp, wcp))

    from concourse.bass_utils import run_bass_kernel_spmd
    res = run_bass_kernel_spmd(nc, in_maps, list(range(N_CORES)))
    outs = [res.results[cix]["out"].reshape(n_img, 1, H, W)
            for cix in range(N_CORES)]
    return np.concatenate(outs, axis=0).astype(np.float32)

